# revision 1
# baseline (speedup 1.0000x reference)
"""Trainium2 Bass kernel for the McSharry-style ECG Euler integrator.

Problem (hardcoded): B=131072 beats, params x[B,15] = interleaved (a,b,theta)
x 5 gaussian waves, v0[B] initial z. 216 Euler steps of
    alpha = 1-|r|; f_x/f_y rotate (x,y);  th = atan2(y,x)
    G = sum_i a_i * dth_i * exp(-dth_i^2/(2 b_i^2)),  dth_i = th - theta_i
    z' = z + H*(-G - (z - z0(t)))
then per-row min/max rescale to [MIN_VAL, MIN_VAL+MAX_VAL... affine].

Key structure: the (x,y) orbit never depends on the batch -> th(t) and z0(t)
are 216-entry constant tables (precomputed on host in exact fp32, like a LUT).
Per row the whole scan collapses to:
    u_t  = H*z0_t + sum_i a'_i * dth_t,i * e_t,i      (a' = -H*a)
    z_t+1 = c*z_t + u_t                               (c = 1-H)

Engine split (all three compute engines ~80-85% busy):
  TensorE: arg_i(t) = -gamma_i*(th_t - theta_i)^2 via expanded quadratic,
           one K=15 fp32 matmul per 128-row tile against a constant basis
           [th^2; th; 1] (coefficients PE-transposed from row-major prep;
           gamma clamped to 1e6 keeps fp32 cancellation harmless).
  ScalarE: one batched exp over all 5 waves (PSUM->SBUF, fp16 out), 3 of 5
           adth = a'*th - a'*theta builds (Identity act with per-partition
           scale/bias), final min/max rescale act.
  VectorE: 2 adth builds, fused product p = adth*e for 5 waves + a seeded
           HZ0 slice in one fp16 tensor_tensor, pairwise-tree accumulate,
           the 216-step Euler recurrence as ONE tensor_tensor_scan
           (state = c*state + u, fp32 state), batched min/max reduce.

Precision: ~6e-4 max rel error vs fp64 (fp16 product chain dominates).
Sharding: pure data-parallel over 8 NeuronCores, 16384 rows each.
"""

import math
import numpy as np

# ---------------------------------------------------------------- constants
B_FULL = 131072
N_CORES = 8
B_SHARD = B_FULL // N_CORES      # 16384
NT = 216                         # time steps
NW = 5                           # gaussian waves
P = 128                          # partitions
NTILES_FULL = B_SHARD // P       # 128 row-tiles per core

H = 1.0 / 216.0
A_Z0 = 0.005
F2 = 0.25
OMEGA = 2.0 * math.pi
X0 = -0.417750770388669
Y0 = -0.9085616622823985
MIN_VAL = -0.01563
MAX_VAL = 0.042557
SQRT2 = math.sqrt(2.0)
SG_CLAMP = 1e19                  # keep 1/(sqrt2*b) finite when b == 0
GMAX = 1e6                       # gamma clamp for the expanded-quadratic arg


def _host_tables():
    """Replicate the reference's fp32 (x,y) Euler orbit -> th, z0 tables."""
    h = np.float32(H)
    om = np.float32(OMEGA)
    one = np.float32(1.0)
    x = np.float32(X0)
    y = np.float32(Y0)
    th = np.empty(NT, np.float32)
    for k in range(NT):
        th[k] = np.arctan2(y, x)
        r = np.sqrt(x * x + y * y)
        alpha = one - r
        fx = alpha * x - om * y
        fy = alpha * y + om * x
        x = x + h * fx
        y = y + h * fy
    t = np.arange(NT, dtype=np.float32) / np.float32(216.0)
    z0 = np.float32(A_Z0) * np.sin(np.float32(2.0 * math.pi * F2) * t)
    return th, z0


def _build_program(ntiles=NTILES_FULL):
    import concourse.bacc as bacc
    import concourse.tile as tile
    from concourse import mybir

    f32 = mybir.dt.float32
    f16 = mybir.dt.float16
    Act = mybir.ActivationFunctionType
    Op = mybir.AluOpType
    X = mybir.AxisListType.X

    rows = ntiles * P
    npar = ntiles * NW
    GB = 8 if ntiles % 8 == 0 else (4 if ntiles % 4 == 0 else 1)
    assert ntiles % GB == 0

    NVW = 2                             # adth waves computed on VectorE
    nc = bacc.Bacc("TRN2", target_bir_lowering=False, debug=False,
                   num_devices=N_CORES)

    theta_d = nc.declare_dram_parameter("theta_t", [P, npar], f32, isOutput=False)
    a_d = nc.declare_dram_parameter("a_t", [P, npar], f32, isOutput=False)
    b_d = nc.declare_dram_parameter("b_t", [P, npar], f32, isOutput=False)
    v0_d = nc.declare_dram_parameter("v0_t", [P, ntiles], f32, isOutput=False)
    thb_d = nc.declare_dram_parameter("th_b", [P, NT], f32, isOutput=False)
    hz0_d = nc.declare_dram_parameter("hz0_b", [P, NT], f32, isOutput=False)
    basis_d = nc.declare_dram_parameter("basis", [P, NW * NT], f32, isOutput=False)
    id_d = nc.declare_dram_parameter("ident", [P, P], f32, isOutput=False)
    out_d = nc.declare_dram_parameter("out", [rows, NT], f32, isOutput=True)
    ngq = (ntiles + 2) // 3             # 3 lhsT slots per 128-partition block

    with tile.TileContext(nc) as tc:
        with tc.tile_pool(name="consts", bufs=1) as consts, \
             tc.tile_pool(name="work", bufs=6) as work, \
             tc.tile_pool(name="zpool", bufs=3) as zpool, \
             tc.tile_pool(name="outp", bufs=4) as outp, \
             tc.tile_pool(name="argp", bufs=2, space="PSUM") as argp, \
             tc.tile_pool(name="trp", bufs=2, space="PSUM") as trp:

            TH = consts.tile([P, NT], f32)
            nc.sync.dma_start(out=TH, in_=thb_d[:, :])
            HZ0 = consts.tile([P, NT], f32)
            nc.sync.dma_start(out=HZ0, in_=hz0_d[:, :])
            HZ0h = consts.tile([P, NT], f16)
            nc.vector.tensor_copy(HZ0h, HZ0)
            CB = consts.tile([P, NT], f32)
            nc.vector.memset(CB, float(np.float32(1.0) - np.float32(H)))
            MINV = consts.tile([P, GB], f32)
            nc.vector.memset(MINV, MIN_VAL)
            BASIS = consts.tile([P, NW * NT], f32)
            nc.sync.dma_start(out=BASIS, in_=basis_d[:, :])
            IDENT = consts.tile([P, P], f32)
            nc.sync.dma_start(out=IDENT, in_=id_d[:, :])

            THETA = consts.tile([P, npar], f32)
            nc.sync.dma_start(out=THETA, in_=theta_d[:, :])
            A = consts.tile([P, npar], f32)
            nc.sync.dma_start(out=A, in_=a_d[:, :])
            Bt = consts.tile([P, npar], f32)
            nc.sync.dma_start(out=Bt, in_=b_d[:, :])
            V0 = consts.tile([P, ntiles], f32)
            nc.sync.dma_start(out=V0, in_=v0_d[:, :])

            # ring tiles: slice 5 pre-seeded (e:HZ0h, adth:1) so the product
            # tree sums 5 waves + HZ0 in one pass
            RING = 6
            e6r_l = []
            adth6r_l = []
            for k in range(RING):
                e6k = consts.tile([P, NW + 1, NT], f16, name=f"e6r{k}")
                nc.vector.tensor_copy(e6k[:, NW, :], HZ0h)
                e6r_l.append(e6k)
                a6k = consts.tile([P, NW + 1, NT], f16, name=f"adth6r{k}")
                nc.vector.memset(a6k[:, NW, :], 1.0)
                adth6r_l.append(a6k)
            p6r_l = [consts.tile([P, NW + 1, NT], f16, name=f"p6r{k}")
                     for k in range(RING)]

            # prep: A1 = -H*a ; C2N = -A1*theta  (adth = A1*th + C2N)
            A1 = consts.tile([P, npar], f32)
            nc.vector.tensor_scalar_mul(A1, A, -H)
            C2N = consts.tile([P, npar], f32)
            nc.vector.scalar_tensor_tensor(C2N, A1, -1.0, THETA, Op.mult, Op.mult)

            # PR: per-row arg-matmul coefficients, laid out for PE transpose.
            # PR[p, g, k]: k in 0..31, rows k'*5+i hold [-gc, 2*gc*th, -gc*th^2]
            PR = consts.tile([P, ntiles, 32], f32)
            th3 = THETA.rearrange("p (g w) -> p g w", w=NW)
            b3 = Bt.rearrange("p (g w) -> p g w", w=NW)
            for i in range(NW):
                gc = work.tile([P, ntiles], f32, tag="gc")
                nc.vector.tensor_mul(gc, b3[:, :, i], b3[:, :, i])
                nc.vector.tensor_scalar_mul(gc, gc, 2.0)
                nc.vector.reciprocal(gc, gc)
                nc.vector.tensor_scalar_min(gc, gc, GMAX)
                nc.vector.tensor_scalar_mul(PR[:, :, 0 * NW + i], gc, -1.0)
                nc.vector.scalar_tensor_tensor(PR[:, :, 1 * NW + i], gc, 2.0,
                                               th3[:, :, i], Op.mult, Op.mult)
                nc.vector.scalar_tensor_tensor(PR[:, :, 2 * NW + i],
                                               PR[:, :, 1 * NW + i], -0.5,
                                               th3[:, :, i], Op.mult, Op.mult)

            # transpose coefficient blocks: LHS[gq][slot*32+k, p] = PR[p, gq*3+slot, k]
            LHS = []
            for gq in range(ngq):
                nslot = min(3, ntiles - gq * 3)
                ptr = trp.tile([P, P], f32, tag="ptr")
                nc.tensor.transpose(
                    ptr[:nslot * 32, :],
                    PR[:, gq * 3:gq * 3 + nslot, :].rearrange("p a b -> p (a b)"),
                    IDENT)
                lhs = consts.tile([P, P], f32, name=f"lhs{gq}")
                nc.scalar.copy(lhs[:nslot * 32, :], ptr[:nslot * 32, :])
                LHS.append(lhs)

            for gb in range(ntiles // GB):
                z4 = zpool.tile([P, GB, NT], f32, tag="z4")
                for j in range(GB):
                    g = gb * GB + j
                    slot = (g % 3) * 32
                    lhs_g = LHS[g // 3][slot:slot + 15, :]
                    bas_g = BASIS[slot:slot + 15, :]
                    argps = argp.tile([P, NW * NT], f32, tag="argps")
                    nc.tensor.matmul(argps[:, 0:512], lhs_g, bas_g[:, 0:512],
                                     start=True, stop=True)
                    nc.tensor.matmul(argps[:, 512:1024], lhs_g, bas_g[:, 512:1024],
                                     start=True, stop=True)
                    nc.tensor.matmul(argps[:, 1024:1080], lhs_g, bas_g[:, 1024:1080],
                                     start=True, stop=True)
                    e6 = e6r_l[g % RING]
                    nc.scalar.activation(
                        e6[:, 0:NW, :].rearrange("p w t -> p (w t)"),
                        argps, Act.Exp)
                    adth6 = adth6r_l[g % RING]
                    for i in range(NW):
                        col = g * NW + i
                        if i < NVW:
                            nc.vector.tensor_scalar(adth6[:, i, :], TH,
                                                    A1[:, col:col + 1],
                                                    C2N[:, col:col + 1],
                                                    Op.mult, Op.add)
                        else:
                            nc.scalar.activation(adth6[:, i, :], TH, Act.Identity,
                                                 bias=C2N[:, col:col + 1],
                                                 scale=A1[:, col:col + 1])
                    p6 = p6r_l[g % RING]
                    nc.vector.tensor_mul(p6.rearrange("p w t -> p (w t)"),
                                         adth6.rearrange("p w t -> p (w t)"),
                                         e6.rearrange("p w t -> p (w t)"))
                    q3 = work.tile([P, 3, NT], f16, tag="q3")
                    nc.vector.tensor_add(q3.rearrange("p w t -> p (w t)"),
                                         p6[:, 0:3, :].rearrange("p w t -> p (w t)"),
                                         p6[:, 3:6, :].rearrange("p w t -> p (w t)"))
                    acc = work.tile([P, NT], f16, tag="acc")
                    nc.vector.tensor_add(acc, q3[:, 0, :], q3[:, 1, :])
                    nc.vector.tensor_add(acc, acc, q3[:, 2, :])
                    nc.vector.tensor_tensor_scan(z4[:, j, :], CB, acc,
                                                 V0[:, g:g + 1], Op.mult, Op.add)

                zmin = work.tile([P, GB], f32, tag="zmin")
                zmax = work.tile([P, GB], f32, tag="zmax")
                nc.vector.tensor_reduce(zmin, z4, axis=X, op=Op.min)
                nc.vector.tensor_reduce(zmax, z4, axis=X, op=Op.max)
                d4 = work.tile([P, GB], f32, tag="d4")
                nc.vector.tensor_sub(d4, zmax, zmin)
                r4 = work.tile([P, GB], f32, tag="r4")
                nc.vector.reciprocal(r4, d4)
                s4 = work.tile([P, GB], f32, tag="s4")
                nc.vector.tensor_scalar_mul(s4, r4, MAX_VAL)
                t4 = work.tile([P, GB], f32, tag="t4")
                nc.vector.tensor_mul(t4, zmin, s4)
                bo4 = work.tile([P, GB], f32, tag="bo4")
                nc.vector.tensor_sub(bo4, MINV, t4)
                o4 = outp.tile([P, GB, NT], f32, tag="o4")
                for j in range(GB):
                    nc.scalar.activation(o4[:, j, :], z4[:, j, :], Act.Identity,
                                         bias=bo4[:, j:j + 1],
                                         scale=s4[:, j:j + 1])
                nc.sync.dma_start(
                    out=out_d[gb * GB * P:(gb + 1) * GB * P, :].rearrange(
                        "(j p) t -> p j t", p=P),
                    in_=o4)

    nc.compile()
    return nc


_PROG_CACHE = {}


def _get_program(ntiles=NTILES_FULL):
    if ntiles not in _PROG_CACHE:
        _PROG_CACHE[ntiles] = _build_program(ntiles)
    return _PROG_CACHE[ntiles]


def _make_in_maps(x, v0, ntiles=NTILES_FULL):
    """Shard + lay out inputs per core.

    Layout: row r = g*128 + p -> partition p, tile g.  theta/a/b land as
    [128, ntiles*5] with column g*5+i; v0 as [128, ntiles]."""
    th, z0 = _host_tables()
    thb = np.broadcast_to(th, (P, NT)).copy()
    hz0 = np.broadcast_to((np.float32(H) * z0), (P, NT)).copy()

    # basis[k'*5+i, i*216+t] = {th^2, th, 1}[k'] for wave i (block-diagonal)
    basis = np.zeros((P, NW * NT), np.float32)
    th2 = (th * th).astype(np.float32)
    for slot in range(3):
        for i in range(NW):
            basis[slot * 32 + 0 * NW + i, i * NT:(i + 1) * NT] = th2
            basis[slot * 32 + 1 * NW + i, i * NT:(i + 1) * NT] = th
            basis[slot * 32 + 2 * NW + i, i * NT:(i + 1) * NT] = 1.0
    ident = np.eye(P, dtype=np.float32)

    x = np.ascontiguousarray(np.asarray(x, dtype=np.float32))
    v0 = np.ascontiguousarray(np.asarray(v0, dtype=np.float32))

    rows = ntiles * P
    in_maps = []
    for c in range(N_CORES):
        xs = x[c * B_SHARD: c * B_SHARD + rows]          # [rows, 15]
        vs = v0[c * B_SHARD: c * B_SHARD + rows]         # [rows]
        # [g, p, 15] -> [p, g, 15]
        xr = xs.reshape(ntiles, P, 15).transpose(1, 0, 2)
        a_t = np.ascontiguousarray(xr[:, :, 0::3]).reshape(P, ntiles * NW)
        b_t = np.ascontiguousarray(xr[:, :, 1::3]).reshape(P, ntiles * NW)
        theta_t = np.ascontiguousarray(xr[:, :, 2::3]).reshape(P, ntiles * NW)
        v0_t = np.ascontiguousarray(vs.reshape(ntiles, P).T)
        in_maps.append({
            "theta_t": np.ascontiguousarray(theta_t),
            "a_t": np.ascontiguousarray(a_t),
            "b_t": np.ascontiguousarray(b_t),
            "v0_t": v0_t,
            "th_b": thb,
            "hz0_b": hz0,
            "basis": basis,
            "ident": ident,
        })
    return in_maps


def kernel_run(x, v0, trace=False, ntiles=NTILES_FULL):
    """Run the bass kernel; returns (out [B,216] f32, BassKernelResults)."""
    from concourse.bass_utils import run_bass_kernel_spmd

    nc = _get_program(ntiles)
    in_maps = _make_in_maps(x, v0, ntiles)
    res = run_bass_kernel_spmd(nc, in_maps, list(range(N_CORES)), trace=trace)
    out = np.concatenate([res.results[c]["out"] for c in range(N_CORES)], axis=0)
    return out, res


def kernel(x, v0):
    out, _ = kernel_run(x, v0)
    return out

